# revision 1
# baseline (speedup 1.0000x reference)
# Trainium2 Bass kernel for nn_BiMambaLayer (BiMamba block: AdaRMSNorm ->
# bidirectional Mamba -> out-proj residual -> AdaRMSNorm -> SwiGLU FFN).
#
# Sharding: 8 cores = 2 directions x 4 batches (core c: dir=c//4, b=c%4).
# Each core runs one direction of one sequence in its local time order
# (dir-1 cores receive time-flipped inputs, so the SPMD program is identical).
# The two directions of a batch meet once: a pair AllReduce of the projected
# mamba branch output in canonical (global) time order. The FFN phase is
# token-split across the pair (each core finishes 512 tokens).
#
# Layout: feature-major everywhere — activations [feature on partitions,
# time on free dim], which PE matmuls need (contraction on partitions).
# The main pipeline runs in two 512-token halves to fit SBUF; the Mamba scan
# chains across halves via per-(e,n) saved states.
import numpy as np
import ml_dtypes

try:
    import ntff_hook_shim  # noqa: F401  (optional, enables trace in dev)
    ntff_hook_shim.install()
except Exception:
    pass

import concourse.bass as bass
import concourse.tile as tile
from concourse.tile import add_dep_helper
from concourse import bacc, mybir
from concourse.bass_utils import run_bass_kernel_spmd
from contextlib import ExitStack

F32 = mybir.dt.float32
BF16 = mybir.dt.bfloat16
AF = mybir.ActivationFunctionType
OP = mybir.AluOpType

D = 512          # d_model
ED = 1024        # d_inner
N = 16           # d_state
R = 32           # dt_rank
DC = 4           # d_conv
FF = 1536        # d_ff
B, L = 4, 1024
EPS = 1e-6
HL = L // 2      # tokens per half / per core in the FFN phase
NCORES = 8
ET = ED // 128   # 8 e-tiles
DT = D // 128    # 4 d-tiles
FT = FF // 128   # 12

_cache = {}


def _rev(ap):
    """Reverse the (last) free dim of a 2D AP."""
    a = list(ap.ap)
    assert len(a) == 2
    stride, n = a[1]
    return bass.AP(tensor=ap.tensor, offset=ap.offset + stride * (n - 1),
                  ap=[a[0], [-stride, n]])


def _bcast_row(src_row):
    """AP that reads a [1, F] SBUF row 128x (replication via stride-0 free dim)."""
    a = list(src_row.ap)
    return bass.AP(tensor=src_row.tensor, offset=src_row.offset,
                  ap=[a[0], [0, 128], a[1]])


def build(debug=False):
    nc = bacc.Bacc("TRN2", target_bir_lowering=False, debug=False,
                   num_devices=NCORES)

    def din(name, shape, dt=F32):
        return nc.dram_tensor(name, shape, dt, kind="ExternalInput").ap()

    ins = {}
    ins["xT"] = din("xT", [D, L])                  # x[b].T, time-flipped for dir1
    ins["xbT"] = din("xbT", [D, L], BF16)          # bf16 copy of xT
    ins["pnT"] = din("pnT", [2, L], BF16)          # phys_norm[b].T (flipped for dir1)
    ins["dirmask"] = din("dirmask", [128, 2])      # col0=1-dir, col1=dir
    for p in ("n1", "n2"):
        ins[p + "_w1T"] = din(p + "_w1T", [2, 2 * D], BF16)
        ins[p + "_b1"] = din(p + "_b1", [2 * D, 1])
        ins[p + "_w2T"] = din(p + "_w2T", [2 * D, 2 * D], BF16)
        ins[p + "_b2"] = din(p + "_b2", [2 * D, 1])
        ins[p + "_sc"] = din(p + "_sc", [D, 1])
    ins["in_wT"] = din("in_wT", [D, 2 * ED], BF16)
    ins["conv_w"] = din("conv_w", [ED, DC])
    ins["conv_b"] = din("conv_b", [ED, 1])
    ins["xp_wT"] = din("xp_wT", [ED, R + 2 * N], BF16)
    ins["dt_wT"] = din("dt_wT", [R, ED], BF16)
    ins["dt_b"] = din("dt_b", [ED, 1])
    ins["A"] = din("A", [ED, N])
    ins["Dsk"] = din("Dsk", [ED, 1])
    ins["WdT"] = din("WdT", [ED, D], BF16)         # (blk_half_dir @ out_w_dir).T
    ins["ls1"] = din("ls1", [D, 1])
    ins["fc1_wT"] = din("fc1_wT", [D, 2 * FF], BF16)
    ins["fc1_b"] = din("fc1_b", [2 * FF, 1])
    ins["fc2_wT"] = din("fc2_wT", [FF, D], BF16)
    ins["ls2"] = din("ls2", [D, 1])
    ins["c2T"] = din("c2T", [1, D], BF16)          # fc2_b as a row

    out_ap = nc.dram_tensor("out", [D, HL], F32, kind="ExternalOutput").ap()
    dbg = {}
    if debug:
        def dout(name, shape, dt=BF16):
            dbg[name] = nc.dram_tensor(name, shape, dt, kind="ExternalOutput").ap()
        dout("h_dbg", [D, L])
        dout("u_dbg", [ED, L])
        dout("sz_dbg", [ED, L])
        dout("dt_dbg", [ED, L])
        dout("bc_dbg", [2 * N, L])
        dout("y2_dbg", [ED, L])
        dout("p_dbg", [D, L])
        dout("x1_dbg", [D, HL], F32)
        dout("h2_dbg", [D, HL])

    with tile.TileContext(nc) as tc, ExitStack() as ctx:
        wpool = ctx.enter_context(tc.tile_pool(name="weights", bufs=1))
        wbig = ctx.enter_context(tc.tile_pool(name="wbig", bufs=2))
        big = ctx.enter_context(tc.tile_pool(name="big", bufs=1))
        slab = ctx.enter_context(tc.tile_pool(name="slab", bufs=2))
        rot = ctx.enter_context(tc.tile_pool(name="rot", bufs=2))
        psum = ctx.enter_context(tc.tile_pool(name="psum", bufs=6, space="PSUM"))
        dram = ctx.enter_context(tc.tile_pool(name="dram", bufs=1, space="DRAM"))

        _dma_rr = [nc.sync, nc.scalar, nc.gpsimd]
        _rr = [0]

        def _wdma(out, in_):
            _dma_rr[_rr[0] % 3].dma_start(out, in_)
            _rr[0] += 1

        def w_big(name, K, M, dt=BF16):
            # big weights share rotating "wslab" slots (freed after last use)
            t = wbig.tile([128, K // 128, M], dt, name=name + "_sb", tag="wslab")
            _wdma(t[:], ins[name].rearrange("(a p) m -> p a m", p=128))
            return t

        def w_perm(name, K, M, dt=BF16):
            t = wpool.tile([128, K // 128, M], dt, name=name + "_sb")
            _wdma(t[:], ins[name].rearrange("(a p) m -> p a m", p=128))
            return t

        def w_vec(name, K, dt=F32):
            t = wpool.tile([128, K // 128, 1], dt, name=name + "_sb")
            _wdma(t[:], ins[name].rearrange("(a p) o -> p a o", p=128))
            return t

        # ---- permanent small weights ----
        pn_sb = wpool.tile([2, L], BF16, name="pn_sb")
        nc.sync.dma_start(pn_sb[:], ins["pnT"][:])
        msk = wpool.tile([128, 2], F32, name="msk_sb")
        nc.sync.dma_start(msk[:], ins["dirmask"][:])
        n1_w1 = wpool.tile([2, 2 * D], BF16, name="n1_w1_sb")
        nc.sync.dma_start(n1_w1[:], ins["n1_w1T"][:])
        n2_w1 = wpool.tile([2, 2 * D], BF16, name="n2_w1_sb")
        nc.sync.dma_start(n2_w1[:], ins["n2_w1T"][:])
        n1_b1 = w_vec("n1_b1", 2 * D)
        n1_b2 = w_vec("n1_b2", 2 * D)
        n1_sc = w_vec("n1_sc", D)
        n2_b1 = w_vec("n2_b1", 2 * D)
        n2_b2 = w_vec("n2_b2", 2 * D)
        n2_sc = w_vec("n2_sc", D)
        conv_w = wpool.tile([128, ET, DC], F32, name="conv_w_sb")
        nc.sync.dma_start(conv_w[:], ins["conv_w"].rearrange("(a p) m -> p a m", p=128))
        conv_b = w_vec("conv_b", ED)
        xp_w = w_perm("xp_wT", ED, R + 2 * N)
        dt_w = wpool.tile([R, ED], BF16, name="dt_w_sb")
        nc.sync.dma_start(dt_w[:], ins["dt_wT"][:])
        dt_b = w_vec("dt_b", ED)
        A_sb = wpool.tile([128, ET, N], F32, name="A_sb")
        nc.sync.dma_start(A_sb[:], ins["A"].rearrange("(a p) m -> p a m", p=128))
        Dsk = w_vec("Dsk", ED)
        Wd = w_perm("WdT", ED, D)
        ls1 = w_vec("ls1", D)
        ls2 = w_vec("ls2", D)

        ones_col = wpool.tile([128, 1], BF16, name="ones_col")
        nc.vector.memset(ones_col[:], 1.0)
        ones_row = wpool.tile([1, HL], BF16, name="ones_row")
        nc.vector.memset(ones_row[:], 1.0)
        c2row = wpool.tile([1, D], BF16, name="c2row")
        nc.sync.dma_start(c2row[:], ins["c2T"][:])
        eps_t = wpool.tile([1, 1], F32, name="eps_t")
        nc.vector.memset(eps_t[:], EPS)


        # persistent cross-phase state
        p_bf = big.tile([128, DT, L], BF16, name="p_bf")      # mamba branch out
        hlast = big.tile([128, ET, N], F32, name="hlast")     # scan carry
        x_half = big.tile([128, DT, HL], F32, name="x_half")  # x for residual
        nc.sync.dma_start(x_half[:],
                          ins["xT"].rearrange("(a p) m -> p a m", p=128)[:, :, 0:HL])
        u_dram = dram.tile([128, ET, L], BF16, name="u_dram")
        sz_dram = dram.tile([128, ET, L], BF16, name="sz_dram")

        # =============== ada_norm helper (feature-major, per token slice) ====
        def ada_norm(xb, ntok, pn_ap, w1, b1, w2, b2, sc, pref):
            """xb: list of DT bf16 APs [128, ntok] -> list of DT bf16
            tiles [128, ntok] (normed)."""
            ms_ps = psum.tile([1, ntok], F32, name=f"{pref}_msps", tag="ps_small", bufs=1)
            for i in range(DT):
                sq = rot.tile([128, ntok], BF16, name=f"{pref}_sq", tag="ada_sq",
                              bufs=2)
                nc.vector.tensor_tensor(sq[:], xb[i], xb[i], op=OP.mult)
                for f in range(ntok // 512):
                    nc.tensor.matmul(ms_ps[:, f * 512:(f + 1) * 512],
                                     ones_col[:], sq[:, f * 512:(f + 1) * 512],
                                     start=(i == 0), stop=(i == DT - 1))
            lnm = rot.tile([1, ntok], F32, name=f"{pref}_lnm", tag="ada_lnm", bufs=1)
            nc.scalar.activation(lnm[:], ms_ps[:], AF.Ln, bias=eps_t[:],
                                 scale=1.0 / D)
            rinv = rot.tile([1, ntok], BF16, name=f"{pref}_rinv", tag="ada_rinv", bufs=1)
            nc.scalar.activation(rinv[:], lnm[:], AF.Exp, bias=0.0, scale=-0.5)
            rb = rot.tile([128, ntok], BF16, name=f"{pref}_rb", tag="ada_rb", bufs=1)
            nc.sync.dma_start(rb[:], _bcast_row(rinv[0:1, :]))
            # cond MLP
            sg = rot.tile([128, 2 * DT, ntok], BF16, name=f"{pref}_sg",
                          tag="ada_sg", bufs=1)
            for m in range(2 * DT):
                for f in range(ntok // 512):
                    ps = psum.tile([128, 512], F32, name=f"{pref}_ps1", tag="ps")
                    nc.tensor.matmul(ps[:, :], w1[:, m * 128:(m + 1) * 128],
                                     pn_ap[:, f * 512:f * 512 + 512],
                                     start=True, stop=True)
                    nc.scalar.activation(sg[:, m, f * 512:f * 512 + 512], ps[:, :],
                                         AF.Silu, bias=b1[:, m], scale=1.0)
            h = [rot.tile([128, ntok], BF16, name=f"{pref}_h{i}",
                          tag=f"ada_h{i}", bufs=1) for i in range(DT)]
            # pairs (gamma m, beta m+DT) so tg tiles can rotate
            for i in range(DT):
                tgp = []
                for mm in (i, DT + i):
                    tg = rot.tile([128, ntok], BF16, name=f"{pref}_tg",
                                  tag="ada_tg", bufs=2)
                    for f in range(ntok // 512):
                        ps = psum.tile([128, 512], F32, name=f"{pref}_ps2", tag="ps")
                        for k in range(2 * DT):
                            nc.tensor.matmul(
                                ps[:, :], w2[:, k, mm * 128:(mm + 1) * 128],
                                sg[:, k, f * 512:f * 512 + 512],
                                start=(k == 0), stop=(k == 2 * DT - 1))
                        _i = nc.scalar.activation(tg[:, f * 512:f * 512 + 512],
                                                  ps[:, :], AF.Tanh,
                                                  bias=b2[:, mm], scale=1.0)
                        if pref == "n11":
                            h2_act.append(_i)
                    tgp.append(tg)
                s1 = rot.tile([128, ntok], BF16, name=f"{pref}_s1", tag="ada_s1",
                              bufs=1)
                nc.vector.tensor_scalar(s1[:], tgp[0][:], 0.5, sc[:, i],
                                        op0=OP.mult, op1=OP.add)
                xr = rot.tile([128, ntok], BF16, name=f"{pref}_xr", tag="ada_xr",
                              bufs=1)
                nc.vector.tensor_tensor(xr[:], xb[i], rb[:, :], op=OP.mult)
                hp = rot.tile([128, ntok], BF16, name=f"{pref}_hp", tag="ada_hp",
                              bufs=1)
                nc.vector.tensor_tensor(hp[:], xr[:], s1[:], op=OP.mult)
                nc.vector.scalar_tensor_tensor(h[i][:], tgp[1][:], 0.5, hp[:],
                                               op0=OP.mult, op1=OP.add)
            return h

        # ==================== main pipeline: two token-halves ================
        # Emission order software-pipelines the halves: half-2's matmul-heavy
        # phases are emitted before half-1's DVE-heavy scan so the Tile
        # scheduler can overlap PE/ACT work with the scan, and ACT table-set
        # usage stays grouped (natlog vs silu).
        S = {0: {}, 1: {}}
        cstash = {}
        h2_act = []   # silu-set ACT ops of half-2's P1/P2 (table-thrash guard)

        def p1(hf):
            t0 = hf * HL
            tsl = slice(t0, t0 + HL)
            xb1 = rot.tile([128, DT, HL], BF16, name=f"xb1_{hf}", tag="xb1",
                           bufs=1)
            nc.sync.dma_start(
                xb1[:], ins["xbT"].rearrange("(a p) m -> p a m", p=128)[:, :, tsl])
            h1 = ada_norm([xb1[:, i, :] for i in range(DT)], HL,
                          pn_sb[:, tsl], n1_w1, n1_b1, n1_w2, n1_b2, n1_sc,
                          f"n1{hf}")
            if debug:
                hr = dbg["h_dbg"].rearrange("(a p) m -> p a m", p=128)
                for i in range(DT):
                    nc.sync.dma_start(hr[:, i, tsl], h1[i][:])
            S[hf]["h1"] = h1

        def p2(hf):
            t0 = hf * HL
            tsl = slice(t0, t0 + HL)
            h1 = S[hf]["h1"]
            cstash_new = rot.tile([128, ET, DC - 1], BF16, name=f"cstash{hf}",
                                  tag="cstash", bufs=2)
            for m in range(ET):   # xs rows; interleave z rows as m+ET
                xs = rot.tile([128, DC - 1 + HL], BF16, name=f"xs{hf}", tag="xs",
                              bufs=2)
                ps = psum.tile([128, 512], F32, name="p2ps", tag="ps")
                for k in range(DT):
                    nc.tensor.matmul(ps[:, :], in_w[:, k, m * 128:(m + 1) * 128],
                                     h1[k][:], start=(k == 0), stop=(k == DT - 1))
                nc.scalar.copy(xs[:, DC - 1:], ps[:, :])
                if hf == 0:
                    nc.vector.memset(xs[:, 0:DC - 1], 0.0)
                else:
                    nc.vector.tensor_copy(xs[:, 0:DC - 1], cstash[0][:, m, :])
                nc.vector.tensor_copy(cstash_new[:, m, :], xs[:, HL:HL + DC - 1])
                acc = rot.tile([128, HL], BF16, name="cv_acc", tag="cv_acc")
                nc.vector.tensor_scalar(acc[:], xs[:, 0:HL], conv_w[:, m, 0:1],
                                        None, op0=OP.mult)
                for k in range(1, DC):
                    t2 = rot.tile([128, HL], BF16, name="cv_t", tag="cv_t")
                    nc.vector.tensor_scalar(t2[:], xs[:, k:k + HL],
                                            conv_w[:, m, k:k + 1], None,
                                            op0=OP.mult)
                    acc2 = rot.tile([128, HL], BF16, name="cv_acc", tag="cv_acc")
                    nc.vector.tensor_tensor(acc2[:], acc[:], t2[:], op=OP.add)
                    acc = acc2
                u_t = rot.tile([128, HL], BF16, name="u_t", tag="u_t", bufs=2)
                _i = nc.scalar.activation(u_t[:], acc[:], AF.Silu,
                                          bias=conv_b[:, m], scale=1.0)
                if hf == 1:
                    h2_act.append(_i)
                nc.sync.dma_start(u_dram[:, m, tsl], u_t[:])
                ps2 = psum.tile([128, 512], F32, name="p2ps2", tag="ps")
                for k in range(DT):
                    nc.tensor.matmul(ps2[:, :],
                                     in_w[:, k, (ET + m) * 128:(ET + m + 1) * 128],
                                     h1[k][:], start=(k == 0), stop=(k == DT - 1))
                sz_t = rot.tile([128, HL], BF16, name="sz_t", tag="sz_t", bufs=2)
                _i = nc.scalar.activation(sz_t[:], ps2[:, :], AF.Silu)
                if hf == 1:
                    h2_act.append(_i)
                nc.gpsimd.dma_start(sz_dram[:, m, tsl], sz_t[:])
            cstash[hf] = cstash_new
            if debug:
                nc.gpsimd.dma_start(
                    dbg["u_dbg"].rearrange("(a p) m -> p a m", p=128)[:, :, tsl],
                    u_dram[:, :, tsl])
                nc.gpsimd.dma_start(
                    dbg["sz_dbg"].rearrange("(a p) m -> p a m", p=128)[:, :, tsl],
                    sz_dram[:, :, tsl])

        def p3(hf):
            t0 = hf * HL
            tsl = slice(t0, t0 + HL)
            dbc_bf = rot.tile([R + 2 * N, HL], BF16, name=f"dbc{hf}", tag="dbc",
                              bufs=2)
            ps = psum.tile([R + 2 * N, 512], F32, name="p3ps", tag="ps_small2",
                           bufs=1)
            for k in range(ET):
                uk = rot.tile([128, HL], BF16, name="uk3", tag="uk3", bufs=3)
                nc.sync.dma_start(uk[:], u_dram[:, k, tsl])
                nc.tensor.matmul(ps[:, :], xp_w[:, k, :], uk[:],
                                 start=(k == 0), stop=(k == ET - 1))
            nc.scalar.copy(dbc_bf[:], ps[:, :])
            if debug:
                nc.sync.dma_start(dbg["bc_dbg"][:, tsl], dbc_bf[R:R + 2 * N, :])
            S[hf]["dbc"] = dbc_bf

        def p4(hf):
            t0 = hf * HL
            tsl = slice(t0, t0 + HL)
            dbc_bf = S[hf]["dbc"]
            dt_bf = slab.tile([128, ET, HL], BF16, name=f"dt{hf}", tag="dt",
                              bufs=2)
            dtu = slab.tile([128, ET, HL], BF16, name=f"dtu{hf}", tag="dtu",
                            bufs=1)
            for e in range(ET):
                ps = psum.tile([128, 512], F32, name="p4ps", tag="ps")
                nc.tensor.matmul(ps[:, :], dt_w[:, e * 128:(e + 1) * 128],
                                 dbc_bf[0:R, :], start=True, stop=True)
                ex = rot.tile([128, HL], BF16, name="sp_ex", tag="sp_ex")
                nc.scalar.activation(ex[:], ps[:, :], AF.Exp, bias=dt_b[:, e],
                                     scale=1.0)
                nc.scalar.activation(dt_bf[:, e, :], ex[:], AF.Ln, bias=1.0,
                                     scale=1.0)
                uk = rot.tile([128, HL], BF16, name="uk4", tag="uk3", bufs=3)
                nc.sync.dma_start(uk[:], u_dram[:, e, tsl])
                nc.vector.tensor_tensor(dtu[:, e, :], dt_bf[:, e, :], uk[:],
                                        op=OP.mult)
                if debug:
                    nc.sync.dma_start(
                        dbg["dt_dbg"].rearrange("(a p) m -> p a m", p=128)[:, e, tsl],
                        dt_bf[:, e, :])
            S[hf]["dt"] = dt_bf
            S[hf]["dtu"] = dtu

        def p5(hf):
            dbc_bf = S[hf]["dbc"]
            dt_bf = S[hf]["dt"]
            dtu = S[hf]["dtu"]
            y2 = slab.tile([128, ET, HL], BF16, name=f"y2{hf}", tag="y2",
                           bufs=1)
            for n in range(N):
                Bb = rot.tile([128, HL], BF16, name="Bb", tag="Bb", bufs=2)
                Cb = rot.tile([128, HL], BF16, name="Cb", tag="Cb", bufs=2)
                nc.sync.dma_start(Bb[:], _bcast_row(dbc_bf[R + n:R + n + 1, :]))
                nc.gpsimd.dma_start(Cb[:], _bcast_row(dbc_bf[R + N + n:R + N + n + 1, :]))
                for e in range(ET):
                    dA = rot.tile([128, HL], BF16, name="dA", tag="dA", bufs=3)
                    _di = nc.scalar.activation(dA[:], dt_bf[:, e, :], AF.Exp,
                                               bias=0.0,
                                               scale=A_sb[:, e, n:n + 1])
                    if hf == 0 and n == 0 and e == 0:
                        for _a in h2_act:
                            add_dep_helper(_a.ins, _di.ins, sync=False,
                                           reason="act-table-set grouping")
                    dBu = rot.tile([128, HL], BF16, name="dBu", tag="dBu", bufs=3)
                    nc.vector.tensor_tensor(dBu[:], dtu[:, e, :], Bb[:], op=OP.mult)
                    hs = rot.tile([128, HL], BF16, name="hs", tag="hs", bufs=3)
                    if hf == 0:
                        nc.vector.tensor_tensor_scan(hs[:], dA[:], dBu[:], 0.0,
                                                     op0=OP.mult, op1=OP.add)
                        nc.scalar.copy(hlast[:, e, n:n + 1],
                                       hs[:, HL - 1:HL])
                    else:
                        nc.vector.tensor_tensor_scan(hs[:], dA[:], dBu[:],
                                                     hlast[:, e, n:n + 1],
                                                     op0=OP.mult, op1=OP.add)
                    if n == 0:
                        nc.vector.tensor_tensor(y2[:, e, :], hs[:], Cb[:],
                                                op=OP.mult)
                    else:
                        m_t = rot.tile([128, HL], BF16, name="m_t", tag="m_t",
                                       bufs=3)
                        nc.vector.tensor_tensor(m_t[:], hs[:], Cb[:], op=OP.mult)
                        nc.vector.tensor_tensor(y2[:, e, :], y2[:, e, :], m_t[:],
                                                op=OP.add)
            S[hf]["y2"] = y2

        def p6(hf):
            t0 = hf * HL
            tsl = slice(t0, t0 + HL)
            y2 = S[hf]["y2"]
            for e in range(ET):
                uk = rot.tile([128, HL], BF16, name="uk6", tag="uk3", bufs=3)
                nc.sync.dma_start(uk[:], u_dram[:, e, tsl])
                szk = rot.tile([128, HL], BF16, name="szk", tag="szk", bufs=2)
                nc.gpsimd.dma_start(szk[:], sz_dram[:, e, tsl])
                ud = rot.tile([128, HL], BF16, name="ud", tag="ud")
                nc.vector.scalar_tensor_tensor(ud[:], uk[:], Dsk[:, e],
                                               y2[:, e, :], op0=OP.mult, op1=OP.add)
                nc.vector.tensor_tensor(y2[:, e, :], ud[:], szk[:], op=OP.mult)
            if debug:
                nc.sync.dma_start(
                    dbg["y2_dbg"].rearrange("(a p) m -> p a m", p=128)[:, :, tsl],
                    y2[:])

        def p7(hf):
            t0 = hf * HL
            tsl = slice(t0, t0 + HL)
            y2 = S[hf]["y2"]
            for m in range(DT):
                ps = psum.tile([128, 512], F32, name="p7ps", tag="ps")
                for k in range(ET):
                    nc.tensor.matmul(ps[:, :], Wd[:, k, m * 128:(m + 1) * 128],
                                     y2[:, k, :], start=(k == 0), stop=(k == ET - 1))
                nc.scalar.copy(p_bf[:, m, tsl], ps[:, :])

        n1_w2 = w_big("n1_w2T", 2 * D, 2 * D)
        in_w = w_big("in_wT", D, 2 * ED)
        p1(0); p2(0); p3(0); p4(0)
        p1(1); p2(1); p3(1)
        p5(0); p6(0); p7(0)
        p4(1); p5(1); p6(1); p7(1)

        # =============== P8: canonicalize + pair AllReduce ===============
        n2_w2 = w_big("n2_w2T", 2 * D, 2 * D)   # prefetch during collective
        cc_in = dram.tile([128, DT, L], BF16, name="cc_in")
        cc_out = dram.tile([128, DT, L], BF16, name="cc_out")
        for m in range(DT):
            t0_ = rot.tile([128, L], BF16, name="pc_t0", tag="pc_t0", bufs=1)
            nc.vector.tensor_scalar(t0_[:], p_bf[:, m, :], msk[:, 0:1], None,
                                    op0=OP.mult)
            pc = rot.tile([128, L], BF16, name="pc", tag="pc", bufs=1)
            nc.vector.scalar_tensor_tensor(pc[:], _rev(p_bf[:, m, :]), msk[:, 1:2],
                                           t0_[:], op0=OP.mult, op1=OP.add)
            nc.sync.dma_start(cc_in[:, m, :], pc[:])
        nc.gpsimd.collective_compute(
            "AllReduce", OP.add,
            replica_groups=[[0, 4], [1, 5], [2, 6], [3, 7]],
            ins=[cc_in.opt()], outs=[cc_out.opt()])
        pg = slab.tile([128, DT, L], BF16, name="pg_sb", tag="y2", bufs=1)
        nc.sync.dma_start(pg[:], cc_out[:])
        if debug:
            nc.sync.dma_start(dbg["p_dbg"].rearrange("(a p) m -> p a m", p=128),
                              pg[:])

        # =============== P9: select local half + residual ===============
        x1 = [rot.tile([128, HL], F32, name=f"x1_{m}", tag=f"x1_{m}", bufs=1)
              for m in range(DT)]
        for m in range(DT):
            t0_ = rot.tile([128, HL], BF16, name="sel_t0", tag="sel_t0", bufs=1)
            nc.vector.tensor_scalar(t0_[:], pg[:, m, 0:HL], msk[:, 0:1], None,
                                    op0=OP.mult)
            psel = rot.tile([128, HL], BF16, name="psel", tag="psel", bufs=1)
            rev_half = _rev(pg[:, m, :])
            rev_half = bass.AP(tensor=rev_half.tensor, offset=rev_half.offset,
                               ap=[list(rev_half.ap)[0], [-1, HL]])
            nc.vector.scalar_tensor_tensor(psel[:], rev_half, msk[:, 1:2], t0_[:],
                                           op0=OP.mult, op1=OP.add)
            nc.vector.scalar_tensor_tensor(x1[m][:], psel[:], ls1[:, m],
                                           x_half[:, m, :], op0=OP.mult, op1=OP.add)
        if debug:
            x1r = dbg["x1_dbg"].rearrange("(a p) m -> p a m", p=128)
            for m in range(DT):
                nc.sync.dma_start(x1r[:, m, :], x1[m][:])

        # =============== P10: ada_norm 2 ===============
        x1b = rot.tile([128, DT, HL], BF16, name="x1b", tag="xb1", bufs=1)
        for m in range(DT):
            nc.vector.tensor_copy(x1b[:, m, :], x1[m][:])
        h2 = ada_norm([x1b[:, m, :] for m in range(DT)], HL, pn_sb[:, 0:HL],
                      n2_w1, n2_b1, n2_w2, n2_b2, n2_sc, "m2")
        if debug:
            h2r = dbg["h2_dbg"].rearrange("(a p) m -> p a m", p=128)
            for m in range(DT):
                nc.sync.dma_start(h2r[:, m, :], h2[m][:])

        # =============== P11: SwiGLU FFN ===============
        fc1_w = w_big("fc1_wT", D, 2 * FF)   # [128, 4, 3072]
        fc1_b = w_vec("fc1_b", 2 * FF)
        fc2_w = w_big("fc2_wT", FF, D)
        sg2 = slab.tile([128, FT, HL], BF16, name="sg2_sb", tag="dt", bufs=2)
        gv = slab.tile([128, FT, HL], BF16, name="gv_sb", tag="dtu", bufs=1)
        for m in range(2 * FT):
            ps = psum.tile([128, 512], F32, name="p11ps", tag="ps")
            for k in range(DT):
                nc.tensor.matmul(ps[:, :], fc1_w[:, k, m * 128:(m + 1) * 128],
                                 h2[k][:], start=(k == 0), stop=(k == DT - 1))
            if m < FT:
                nc.scalar.activation(sg2[:, m, :], ps[:, :], AF.Silu,
                                     bias=fc1_b[:, m], scale=1.0)
            else:
                vv = rot.tile([128, HL], BF16, name="vv", tag="vv", bufs=1)
                nc.scalar.activation(vv[:], ps[:, :], AF.Identity,
                                     bias=fc1_b[:, m], scale=1.0)
                nc.vector.tensor_tensor(gv[:, m - FT, :], sg2[:, m - FT, :], vv[:],
                                        op=OP.mult)
        out_sb = slab.tile([128, DT, HL], F32, name="out_sb", tag="dtu", bufs=1)
        for m in range(DT):
            ps = psum.tile([128, 512], F32, name="p12ps", tag="ps")
            for k in range(FT):
                nc.tensor.matmul(ps[:, :], fc2_w[:, k, m * 128:(m + 1) * 128],
                                 gv[:, k, :], start=(k == 0), stop=False)
            nc.tensor.matmul(ps[:, :], c2row[:, m * 128:(m + 1) * 128],
                             ones_row[:], start=False, stop=True)
            nc.vector.scalar_tensor_tensor(out_sb[:, m, :], ps[:, :], ls2[:, m],
                                           x1[m][:], op0=OP.mult, op1=OP.add)
        nc.sync.dma_start(out_ap.rearrange("(a p) m -> p a m", p=128), out_sb[:])

    nc.compile()
    return nc, dbg


def _prep_inputs(inputs):
    """Host-side: per-core input dicts."""
    f32 = np.float32
    bf = ml_dtypes.bfloat16
    x = np.asarray(inputs["x"], f32)
    pn = np.asarray(inputs["phys_norm"], f32)
    blk_w = np.asarray(inputs["blk_w"], f32)
    in_maps = []
    for c in range(NCORES):
        d, b = c // 4, c % 4
        xb = x[b] if d == 0 else x[b, ::-1]
        pnb = pn[b] if d == 0 else pn[b, ::-1]
        Wd = blk_w[:, d * D:(d + 1) * D] @ np.asarray(inputs["m_out_w"][d], f32)
        m = {
            "xT": np.ascontiguousarray((xb + (np.asarray(inputs["ls1"], f32)
                   * np.asarray(inputs["blk_b"], f32))[None, :]).T),
            "xbT": np.ascontiguousarray(xb.T).astype(bf),
            "pnT": np.ascontiguousarray(pnb.T).astype(bf),
            "dirmask": np.tile(np.array([[1.0 - d, float(d)]], f32), (128, 1)),
            "in_wT": np.ascontiguousarray(inputs["m_in_w"][d].T).astype(bf),
            "conv_w": np.asarray(inputs["m_conv_w"][d], f32),
            "conv_b": np.asarray(inputs["m_conv_b"][d], f32).reshape(ED, 1),
            "xp_wT": np.ascontiguousarray(inputs["m_xproj_w"][d].T).astype(bf),
            "dt_wT": np.ascontiguousarray(inputs["m_dt_w"][d].T).astype(bf),
            "dt_b": np.asarray(inputs["m_dt_b"][d], f32).reshape(ED, 1),
            "A": (-np.exp(np.asarray(inputs["m_A_log"][d], f32))),
            "Dsk": np.asarray(inputs["m_D"][d], f32).reshape(ED, 1),
            "WdT": np.ascontiguousarray(Wd.T).astype(bf),
            "ls1": np.asarray(inputs["ls1"], f32).reshape(D, 1),

            "fc1_wT": np.ascontiguousarray(inputs["fc1_w"].T).astype(bf),
            "fc1_b": np.asarray(inputs["fc1_b"], f32).reshape(2 * FF, 1),
            "fc2_wT": np.ascontiguousarray(inputs["fc2_w"].T).astype(bf),
            "ls2": np.asarray(inputs["ls2"], f32).reshape(D, 1),
            "c2T": np.asarray(inputs["fc2_b"], f32).reshape(1, D).astype(bf),
        }
        for p in ("n1", "n2"):
            m[p + "_w1T"] = np.ascontiguousarray(inputs[p + "_w1"].T).astype(bf)
            m[p + "_b1"] = np.asarray(inputs[p + "_b1"], f32).reshape(2 * D, 1)
            m[p + "_w2T"] = np.ascontiguousarray(inputs[p + "_w2"].T).astype(bf)
            m[p + "_b2"] = np.asarray(inputs[p + "_b2"], f32).reshape(2 * D, 1)
            m[p + "_sc"] = np.asarray(inputs[p + "_scale"], f32).reshape(D, 1)
        in_maps.append(m)
    return in_maps


def run(inputs, debug=False, trace=False):
    key = ("dbg" if debug else "lean")
    if key not in _cache:
        _cache[key] = build(debug=debug)
    nc, dbg = _cache[key]
    in_maps = _prep_inputs(inputs)
    res = run_bass_kernel_spmd(nc, in_maps, core_ids=list(range(NCORES)),
                               trace=trace)
    out = np.zeros((B, L, D), np.float32)
    for c in range(NCORES):
        d, b = c // 4, c % 4
        o = res.results[c]["out"]  # [D, HL]
        if d == 0:
            out[b, 0:HL] = o.T
        else:
            out[b, HL:L] = o[:, ::-1].T
    return out, res


def kernel(**inputs):
    out, _ = run(inputs, debug=False, trace=False)
    return out

